# revision 3
# baseline (speedup 1.0000x reference)
"""Trainium2 Bass kernel for 12-head causal multi-head attention.

Problem: B=8, T=1024, C=768, H=12, HS=64, fp32.
Sharding: data-parallel over batch — core b computes batch element b.

Numerics/performance design (vs the f32r original):
  - q/k/v projections run as fp8e4 DoubleRow matmuls (0.5 cy/col, 256
    contraction per instruction) with a first-order residual expansion:
    (x8+dx8)(w8+dw8) ~ x8 w8 + dx8 w8 + x8 dw8 gives ~0.2% error at 75%
    of the fp16 cycle cost. Weights are pre-scaled by 32 on the host so
    their fp8 quantization stays in the normal range; the descale folds
    into the exp scale and the host-side Wproj scaling.
  - scores are one-sided-compensated fp8 DoubleRow: k is stored as
    {k8, dk8=k-k8} and q as plain fp8, so one DoubleRow instruction per
    (head, k-tile) computes (k8+dk8)*q8 = k*q8 at half the fp16 cost with
    only q-side quantization error (~1e-2 final rel err, budget 2e-2).
  - PV and the output projection stay fp16 (their operand error flows
    straight to the output); rowsum/reciprocal stay f32r (1/rowsum
    underflows fp16 subnormals); y is emitted fp16, bias added on host.
  - scheduling: per-pair software pipelining with half-group qkT fillers
    dripped into exp-wait bubbles, diagonal (masked) k-tiles ordered so
    the last PV of each q-window never waits on the exp->mask chain,
    first projection half dripped into the last pair's attention, DMAs
    kept off the gpsimd queue (software-DGE costs ~1us of Pool each).
"""

import os
import numpy as np

B, T, C = 8, 1024, 768
H, HS = 12, 64
WSCALE = 32.0  # fp8 weight pre-scale; descaled in exp scale / host wp
NPAIR = 6  # head pairs (2 heads of 64 -> 128 partitions)
NCK = 6    # contraction chunks of 128 over C
NT = 8     # token tiles of 128

LAST_EXEC_NS = None
LAST_RESULTS = None

_cached_nc = None


def _build_nc():
    import concourse.bass as bass
    import concourse.mybir as mybir
    import concourse.tile as tile
    from concourse import bacc
    from concourse.masks import make_upper_triangular

    f32 = mybir.dt.float32
    f32r = mybir.dt.float32r
    f16 = mybir.dt.float16
    f8 = mybir.dt.float8e4
    AF = mybir.ActivationFunctionType

    nc = bacc.Bacc("TRN2", target_bir_lowering=False, debug=False, num_devices=8)

    xT_d = nc.dram_tensor("xT", [2, C, T], f8, kind="ExternalInput")
    wqk_d = nc.dram_tensor(
        "wqk", [NPAIR, 128, 2, NCK, 256], f8, kind="ExternalInput"
    )
    wv_d = nc.dram_tensor("wv", [2, C, C], f8, kind="ExternalInput")
    wp_d = nc.dram_tensor("wp", [NPAIR, 128, C], f16, kind="ExternalInput")
    y_d = nc.dram_tensor("y", [T, C], f16, kind="ExternalOutput")

    with tile.TileContext(nc) as tc:
        with (
            tc.tile_pool(name="const", bufs=1) as const,
            tc.tile_pool(name="work", bufs=2) as work,
            tc.tile_pool(name="ppool", bufs=4) as ppool,
            tc.tile_pool(name="opool", bufs=1) as opool,
            tc.tile_pool(name="ps1", bufs=2, space="PSUM") as ps1,
        ):
            # ---------- resident inputs / constants ----------
            # wqkt for pair 0 first (the first qkT matmuls need it), split in
            # two so the first chunk lands fast; everything else on the sync
            # and scalar queues only — gpsimd DMAs run as software-DGE and
            # burn ~1us of Pool engine each
            wqkt0 = work.tile([128, 2, NCK, 256], f8, tag="wqkt", bufs=3,
                              name="wqkt")
            nc.sync.dma_start(out=wqkt0[:, 0, :, :], in_=wqk_d[0, :, 0])
            nc.scalar.dma_start(out=wqkt0[:, 1, :, :], in_=wqk_d[0, :, 1])
            # x8 / dx8 as single tiles with the contraction-chunk pairs
            # adjacent in the free dim (DoubleRow rhs wants [128, 2, N])
            xt8 = const.tile([128, NCK, T], f8, name="xt8")
            dxt8 = const.tile([128, NCK, T], f8, name="dxt8")
            wv8 = const.tile([128, NCK, C], f8, name="wv8")
            dwv8 = const.tile([128, NCK, C], f8, name="dwv8")

            def _xap(base, j):
                return bass.AP(
                    tensor=xT_d, offset=base * C * T + 2 * j * 128 * T,
                    ap=[[T, 128], [128 * T, 2], [1, T]],
                )

            def _wvap(base, j):
                return bass.AP(
                    tensor=wv_d, offset=base * C * C + 2 * j * 128 * C,
                    ap=[[C, 128], [128 * C, 2], [1, C]],
                )

            for j in range(3):
                eng = nc.sync if j % 2 == 0 else nc.scalar
                eng.dma_start(out=xt8[:, 2 * j:2 * j + 2, :], in_=_xap(0, j))
                eng2 = nc.scalar if j % 2 == 0 else nc.sync
                eng2.dma_start(out=dxt8[:, 2 * j:2 * j + 2, :], in_=_xap(1, j))
            for j in range(3):
                nc.sync.dma_start(out=wv8[:, 2 * j:2 * j + 2, :],
                                  in_=_wvap(0, j))
                nc.sync.dma_start(out=dwv8[:, 2 * j:2 * j + 2, :],
                                  in_=_wvap(1, j))
            U = const.tile([128, 128], f16)
            make_upper_triangular(nc, U[:, :], val=1.0, diag=True)
            ones_f = const.tile([128, 12], f32)
            nc.vector.memset(ones_f, 1.0)
            ones_t = const.tile([128, 128], f32r)
            nc.vector.tensor_copy(
                out=ones_t, in_=ones_f[:, 0:1].broadcast_to([128, 128])
            )

            # v in token-major layout: per k-tile, 12 heads x (64 cols of v | 1)
            v_all = const.tile([128, NT, H * 65], f16)
            v_heads = v_all.rearrange("p k (h c) -> p k h c", h=H)
            for kt in range(NT):
                nc.vector.tensor_copy(
                    out=v_heads[:, kt, :, 64:65],
                    in_=ones_f.rearrange("p (h o) -> p h o", o=1),
                )

            outTs = [
                opool.tile([128, T], f16, tag=f"outT{p}", name=f"outT{p}")
                for p in range(NPAIR)
            ]
            rs3 = [
                opool.tile([128, T], f32r, tag=f"rs{j}", name=f"rs{j}")
                for j in range(3)
            ]

            # ---------- emit helpers (software-pipelined) ----------
            def emit_vproj(kts):
                # v = (x8+dx8)(wv8+dwv8) to first order: three DoubleRow
                # accumulation passes (x8*wv8 + dx8*wv8 + x8*dwv8), each
                # contracting 256 channels per instruction at 0.5 cy/col
                with nc.named_scope("vproj"):
                    for kt in kts:
                        ktsl = slice(kt * 128, (kt + 1) * 128)
                        pv = ps1.tile([128, C], f32, tag="big2", bufs=2,
                                      name="pv")
                        terms = ((xt8, wv8), (xt8, dwv8), (dxt8, wv8))
                        first, last = (0, 0), (2, 2)
                        for ti, (xs, ws) in enumerate(terms):
                            for j in range(3):
                                jsl = slice(2 * j, 2 * j + 2)
                                for n0, n1 in ((0, 512), (512, 768)):
                                    nc.tensor.matmul(
                                        pv[:, n0:n1],
                                        lhsT=xs[:, jsl, ktsl],
                                        rhs=ws[:, jsl, n0:n1],
                                        start=((ti, j) == first),
                                        stop=((ti, j) == last),
                                        perf_mode=(
                                            mybir.MatmulPerfMode.DoubleRow),
                                    )
                        nc.scalar.copy(
                            out=v_heads[:, kt, :, 0:64],
                            in_=pv.rearrange("p (h c) -> p h c", h=H),
                        )

            qkts = {}

            def qkT_group(p, wqkt, qT, kTt, pqks, gi, half):
                # one accumulation group split into two emission halves so
                # filler work drips at ~1.3us granularity; the psum tile is
                # shared across both halves.
                # q/k are computed 3-term compensated in fp8 DoubleRow
                # ((x8+dx8)(w8+dw8) to first order); q is then stored as
                # plain fp8 and k as {k8, dk8=k-k8} so the DoubleRow scores
                # matmul computes (k8+dk8)*q8 = k*q8 at half the per-column
                # cost with only q-side fp8 error
                tch = gi % 2
                tsl = slice(tch * 512, (tch + 1) * 512)
                with nc.named_scope(f"qk{p}"):
                    if half == 0:
                        pqks[gi] = ps1.tile([128, 512], f32, tag="po",
                                            bufs=4, name="pqk")
                    pqk = pqks[gi]
                    wo = (gi // 2) * 128
                    wsl = slice(wo, wo + 128)
                    terms = [(xt8, 0, j) for j in range(3)] + \
                            [(xt8, 1, j) for j in range(3)] + \
                            [(dxt8, 0, j) for j in range(3)]
                    sub = terms[:5] if half == 0 else terms[5:]
                    for (xs, wi, j) in sub:
                        jsl = slice(2 * j, 2 * j + 2)
                        nc.tensor.matmul(
                            pqk,
                            lhsT=wqkt[:, wi, jsl, wsl],
                            rhs=xs[:, jsl, tsl],
                            start=((xs is xt8) and wi == 0 and j == 0),
                            stop=((xs is dxt8) and j == 2),
                            perf_mode=mybir.MatmulPerfMode.DoubleRow,
                        )
                    if half == 1:
                        if gi // 2 == 0:
                            nc.vector.tensor_copy(out=qT[:, tsl], in_=pqk)
                        else:
                            nc.vector.tensor_copy(out=kTt[:, 0, tsl],
                                                  in_=pqk)
                            nc.vector.tensor_sub(
                                kTt[:, 1, tsl], pqk, kTt[:, 0, tsl]
                            )

            def make_qkT_fillers(p, wqkt=None):
                # allocate tiles and launch the weight DMA now; the eight
                # matmul half-groups are emitted later, dripped into exp-wait
                # bubbles of the current pair's attention
                if wqkt is None:
                    wqkt = work.tile([128, 2, NCK, 256], f8, tag="wqkt",
                                     bufs=3, name="wqkt")
                    nc.sync.dma_start(out=wqkt, in_=wqk_d[p])
                qT = work.tile([128, T], f8, tag="qT", bufs=3, name="qT")
                kTt = work.tile([128, 2, T], f8, tag="kTt", bufs=3,
                                name="kTt")
                qkts[p] = (qT, kTt)
                pqks = {}
                return [
                    (lambda gi=gi, half=half: qkT_group(
                        p, wqkt, qT, kTt, pqks, gi, half))
                    for gi in range(4)
                    for half in range(2)
                ]

            for f in make_qkT_fillers(0, wqkt0):
                f()
            fillers = []

            # ---------- weights for proj (load during phase 1) ----------
            wpts = []
            for pp in range(NPAIR):
                wpt = const.tile([128, C], f16, tag=f"wp{pp}", name=f"wp{pp}")
                wpts.append(wpt)

            proj_state = {}

            def proj_piece(tt, half):
                # one half (pairs 0-2 | 3-5) of one token tile's projection;
                # pieces are dripped into the last pair's attention bubbles
                with nc.named_scope("proj"):
                    if half == 0:
                        proj_state[tt] = ps1.tile([128, C], f32, tag="big2",
                                                  bufs=2, name="py")
                    py = proj_state[tt]
                    for p in range(3 * half, 3 * half + 3):
                        for n0, n1 in ((0, 512), (512, 768)):
                            nc.tensor.matmul(
                                py[:, n0:n1],
                                lhsT=outTs[p][:, tt * 128:(tt + 1) * 128],
                                rhs=wpts[p][:, n0:n1],
                                start=(p == 0),
                                stop=(p == NPAIR - 1),
                            )
                    if half == 1:
                        ysb = work.tile([128, C], f16, tag="ysb", bufs=3,
                                        name="ysb")
                        if tt % 2 == 0:
                            nc.vector.tensor_copy(out=ysb, in_=py)
                        else:
                            nc.scalar.copy(out=ysb, in_=py)
                        nc.sync.dma_start(
                            out=y_d[tt * 128:(tt + 1) * 128, :], in_=ysb
                        )

            def emit_proj(tts):
                for tt in tts:
                    proj_piece(tt, 0)
                    proj_piece(tt, 1)

            # ---------- phase 1: attention (qkT/vproj pipelined in) --------
            for p in range(NPAIR):
                qT, kTt = qkts.pop(p)
                if p + 1 < NPAIR:
                    fillers += make_qkT_fillers(p + 1)
                if p == 2:
                    # wp loads: late enough not to fight input DMAs,
                    # early enough to be resident before proj starts
                    for pp in range(NPAIR):
                        nc.sync.dma_start(out=wpts[pp], in_=wp_d[pp])
                with nc.named_scope(f"att{p}"):
                    for qc in range(2):
                        if p == NPAIR - 1 and qc == 1:
                            # all pairs' outT for tokens 0..511 are final:
                            # drip the first half of the projection into this
                            # window's exp-wait bubbles (the last pair has no
                            # next-pair qkT work to fill them)
                            fillers += [
                                (lambda tt=tt, half=half: proj_piece(tt, half))
                                for tt in range(4)
                                for half in range(2)
                            ]
                        nkt = 4 * (qc + 1)
                        po_pair = []
                        for hh in range(2):
                            po = ps1.tile([65, 512], f32, tag="po", bufs=4,
                                          name=f"po{hh}")
                            po_pair.append(po)
                        def scores_exp(kt):
                            # scores pair -> exp -> causal mask for one k-tile
                            ccol = max(0, 128 * kt - 512 * qc)
                            pt = ppool.tile([128, 2, 512], f16, tag="pt",
                                            bufs=6, name="pt")
                            pscr = ps1.tile([128, 2, 512], f32, tag="big2",
                                            bufs=2, name="pscr")
                            nw = 512 - ccol
                            for hh in range(2):
                                nc.tensor.matmul(
                                    pscr[:, hh, ccol:512],
                                    lhsT=kTt[hh * 64:(hh + 1) * 64, :,
                                             kt * 128:(kt + 1) * 128],
                                    rhs=qT[hh * 64:(hh + 1) * 64,
                                           qc * 512 + ccol:(qc + 1) * 512]
                                        .rearrange("p (o n) -> p o n", o=1)
                                        .broadcast_to([64, 2, nw]),
                                    start=True,
                                    stop=True,
                                    perf_mode=mybir.MatmulPerfMode.DoubleRow,
                                )
                            nc.scalar.activation(
                                out=pt[:, :, ccol:512],
                                in_=pscr[:, :, ccol:512],
                                func=AF.Exp,
                                scale=float(HS) ** -0.5 / (WSCALE * WSCALE),
                            )
                            if 128 * kt >= 512 * qc:
                                nc.gpsimd.tensor_mul(
                                    pt[:, :, ccol:ccol + 128],
                                    pt[:, :, ccol:ccol + 128],
                                    U.rearrange("p (o c) -> p o c", o=1)
                                        .broadcast_to([128, 2, 128]),
                                )
                            return pt, ccol

                        # software-pipelined: scores/exp/mask run ahead of
                        # the PV that consumes them. For pair 0 the lookahead
                        # is deep (scores need only qT/kT) and the v
                        # projection is emitted AFTER the prefill, so exp
                        # work fills the wait for the Wv/x input DMAs.
                        look = 5 if p == 0 else 2
                        # process masked (diagonal-crossing) k-tiles first so
                        # the window's last PV never waits on the exp->mask
                        # chain; the first tile in order must have ccol == 0
                        # (its start=True zeroes the full accumulator range)
                        kt_list = [0, 3, 1, 2] if qc == 0 else \
                                  [0, 5, 6, 4, 7, 1, 2, 3]
                        pts = {}
                        for kk in kt_list[:min(look, nkt)]:
                            pts[kk] = scores_exp(kk)
                        if p == 0 and qc == 0:
                            emit_vproj(range(0, 4))
                        if p == 0 and qc == 1:
                            emit_vproj(range(4, NT))
                        if fillers:
                            # independent PE work while the first exp/mask
                            # chain of this q-window completes
                            fillers.pop(0)()
                        for idx, kt in enumerate(kt_list):
                            if idx + look < nkt:
                                pts[kt_list[idx + look]] = scores_exp(
                                    kt_list[idx + look])
                            pt, ccol = pts.pop(kt)
                            for hh in range(2):
                                h = 2 * p + hh
                                nc.tensor.matmul(
                                    po_pair[hh][:, ccol:512],
                                    lhsT=v_all[:, kt, h * 65:(h + 1) * 65],
                                    rhs=pt[:, hh, ccol:512],
                                    start=(idx == 0),
                                    stop=(idx == nkt - 1),
                                )
                            if fillers and idx % 2 == 1:
                                fillers.pop(0)()
                        qsl = slice(qc * 512, (qc + 1) * 512)
                        # reciprocals first (they gate the pr matmuls on PE)
                        for hh in range(2):
                            h = 2 * p + hh
                            with nc.allow_low_precision(
                                reason="1/rowsum at fp32r (12-bit mantissa) "
                                       "costs ~1e-4 relative error"
                            ):
                                nc.vector.reciprocal(
                                    out=rs3[h // 4][(h % 4) * 32:
                                                    (h % 4) * 32 + 1, qsl],
                                    in_=po_pair[hh][64:65, :],
                                )
                        for hh in range(2):
                            # split across engines so both po banks free in
                            # parallel
                            if hh == 0:
                                nc.scalar.copy(
                                    out=outTs[p][0:64, qsl],
                                    in_=po_pair[0][0:64, :],
                                )
                            else:
                                nc.vector.tensor_copy(
                                    out=outTs[p][64:128, qsl],
                                    in_=po_pair[1][0:64, :],
                                )
                        # normalize: broadcast each 1/rowsum row across all
                        # 128 partitions via a K=1 matmul (matmul dst must
                        # start at partition 0), multiply the matching half
                        for hh in range(2):
                            h = 2 * p + hh
                            r0 = (h % 4) * 32
                            pr = ps1.tile([128, 512], f32, tag="po", bufs=4,
                                          name="pr")
                            nc.tensor.matmul(
                                pr,
                                lhsT=ones_t[r0:r0 + 1, :],
                                rhs=rs3[h // 4][r0:r0 + 1, qsl],
                                start=True,
                                stop=True,
                                tile_position=(r0, 0),
                            )
                            nc.vector.tensor_mul(
                                outTs[p][hh * 64:(hh + 1) * 64, qsl],
                                outTs[p][hh * 64:(hh + 1) * 64, qsl],
                                pr[hh * 64:(hh + 1) * 64, :],
                            )

            for f in fillers:
                f()
            fillers = []

            # ---------- phase 2: rest of the output projection ----------
            emit_proj(range(4, NT))

    nc.compile()
    return nc


def get_nc():
    global _cached_nc
    if _cached_nc is None:
        _cached_nc = _build_nc()
    return _cached_nc


def _f8_pair(a):
    """Quantize to fp8 e4m3 with a first-order residual: returns (a8, da8)
    stacked on axis 0 so a8 + da8 ~= a to ~0.2%."""
    import ml_dtypes

    f8 = ml_dtypes.float8_e4m3
    a = np.asarray(a, np.float32)
    a8 = a.astype(f8)
    da8 = (a - a8.astype(np.float32)).astype(f8)
    return np.stack([a8, da8])


def _host_pack(inputs):
    x = np.asarray(inputs["x"], dtype=np.float32)
    Wq = np.asarray(inputs["Wq"], dtype=np.float32)
    Wk = np.asarray(inputs["Wk"], dtype=np.float32)
    Wv = np.asarray(inputs["Wv"], dtype=np.float32)
    Wproj = np.asarray(inputs["Wproj"], dtype=np.float32)

    Wq2 = Wq.transpose(1, 0, 2).reshape(C, C)  # [c, h*HS]
    Wk2 = Wk.transpose(1, 0, 2).reshape(C, C)
    wqk = np.stack(
        [
            np.concatenate(
                [
                    Wq2[:, p * 128:(p + 1) * 128],
                    Wk2[:, p * 128:(p + 1) * 128],
                ],
                axis=1,
            )
            for p in range(NPAIR)
        ]
    )  # [6, 768(c), 256]
    # -> [6, 128(kp), 6(ck), 256] so the on-chip tile loads contiguously
    wqk = np.ascontiguousarray(
        wqk.reshape(NPAIR, NCK, 128, 256).transpose(0, 2, 1, 3)
    ) * WSCALE
    wqk = np.ascontiguousarray(
        _f8_pair(wqk).transpose(1, 2, 0, 3, 4)
    )  # [6, 128(kp), 2, 6, 256]
    wv = _f8_pair(
        np.ascontiguousarray(Wv.transpose(1, 0, 2).reshape(C, C)) * WSCALE
    )  # [2, C, C]
    wp = np.ascontiguousarray(
        Wproj.T.reshape(NPAIR, 128, C) / WSCALE
    ).astype(np.float16)
    shared = {"wqk": wqk, "wv": wv, "wp": wp}
    in_maps = [
        dict(shared, xT=_f8_pair(np.ascontiguousarray(x[b].T)))
        for b in range(B)
    ]
    return in_maps


def kernel(**inputs):
    global LAST_EXEC_NS, LAST_RESULTS
    from concourse.bass_utils import run_bass_kernel_spmd

    nc = get_nc()
    in_maps = _host_pack(inputs)
    trace = bool(int(os.environ.get("KERNEL_TRACE", "0")))
    res = run_bass_kernel_spmd(
        nc, in_maps, core_ids=list(range(B)), trace=trace
    )
    LAST_EXEC_NS = res.exec_time_ns
    LAST_RESULTS = res
    bproj = np.asarray(inputs["bproj"], dtype=np.float32)
    y = np.stack([res.results[b]["y"] for b in range(B)])
    return y.astype(np.float32) + bproj


# revision 5
# speedup vs baseline: 1.9086x; 1.9086x over previous
"""Trainium2 Bass kernel for 12-head causal multi-head attention.

Problem: B=8, T=1024, C=768, H=12, HS=64, fp32.
Sharding: data-parallel over batch — core b computes batch element b.

Numerics/performance design (vs the f32r original):
  - q/k/v projections run as fp8e4 DoubleRow matmuls (0.5 cy/col, 256
    contraction per instruction) with a first-order residual expansion:
    (x8+dx8)(w8+dw8) ~ x8 w8 + dx8 w8 + x8 dw8 gives ~0.2% error at 75%
    of the fp16 cycle cost. Weights are pre-scaled by 32 on the host so
    their fp8 quantization stays in the normal range; the descale folds
    into the exp scale and the host-side Wproj scaling.
  - scores are one-sided-compensated fp8 DoubleRow: k is stored as
    {k8, dk8=k-k8} and q as plain fp8, so one DoubleRow instruction per
    (head, k-tile) computes (k8+dk8)*q8 = k*q8 at half the fp16 cost with
    only q-side quantization error (~1e-2 final rel err, budget 2e-2).
  - PV and the output projection stay fp16 (their operand error flows
    straight to the output); rowsum/reciprocal stay f32r (1/rowsum
    underflows fp16 subnormals); y is emitted fp16, bias added on host.
  - scheduling: per-pair software pipelining with half-group qkT fillers
    dripped into exp-wait bubbles, diagonal (masked) k-tiles ordered so
    the last PV of each q-window never waits on the exp->mask chain,
    first projection half dripped into the last pair's attention, DMAs
    kept off the gpsimd queue (software-DGE costs ~1us of Pool each).
"""

import os
import numpy as np

B, T, C = 8, 1024, 768
H, HS = 12, 64
WSCALE = 32.0  # fp8 weight pre-scale; descaled in exp scale / host wp
NPAIR = 6  # head pairs (2 heads of 64 -> 128 partitions)
NCK = 6    # contraction chunks of 128 over C
NT = 8     # token tiles of 128

LAST_EXEC_NS = None
LAST_RESULTS = None

_cached_nc = None


def _build_nc():
    import concourse.bass as bass
    import concourse.mybir as mybir
    import concourse.tile as tile
    from concourse import bacc
    from concourse.masks import make_upper_triangular

    f32 = mybir.dt.float32
    f32r = mybir.dt.float32r
    f16 = mybir.dt.float16
    f8 = mybir.dt.float8e4
    AF = mybir.ActivationFunctionType

    nc = bacc.Bacc("TRN2", target_bir_lowering=False, debug=False, num_devices=8)

    xT_d = nc.dram_tensor("xT", [2, C, T], f8, kind="ExternalInput")
    wqk_d = nc.dram_tensor(
        "wqk", [NPAIR, 128, 2, NCK, 256], f8, kind="ExternalInput"
    )
    wv_d = nc.dram_tensor("wv", [2, C, C], f8, kind="ExternalInput")
    wp_d = nc.dram_tensor("wp", [NPAIR, 128, C], f16, kind="ExternalInput")
    y_d = nc.dram_tensor("y", [T, C], f16, kind="ExternalOutput")

    with tile.TileContext(nc) as tc:
        with (
            tc.tile_pool(name="const", bufs=1) as const,
            tc.tile_pool(name="work", bufs=2) as work,
            tc.tile_pool(name="ppool", bufs=4) as ppool,
            tc.tile_pool(name="opool", bufs=1) as opool,
            tc.tile_pool(name="ps1", bufs=2, space="PSUM") as ps1,
            tc.tile_pool(name="dscr", bufs=2, space="DRAM") as dscr,
        ):
            # ---------- resident inputs / constants ----------
            # wqkt for pair 0 first (the first qkT matmuls need it), split in
            # two so the first chunk lands fast; everything else on the sync
            # and scalar queues only — gpsimd DMAs run as software-DGE and
            # burn ~1us of Pool engine each
            wqkt0 = work.tile([128, 2, NCK, 256], f8, tag="wqkt", bufs=3,
                              name="wqkt")
            nc.sync.dma_start(out=wqkt0[:, 0, :, :], in_=wqk_d[0, :, 0])
            nc.scalar.dma_start(out=wqkt0[:, 1, :, :], in_=wqk_d[0, :, 1])
            # x8 / dx8 as single tiles with the contraction-chunk pairs
            # adjacent in the free dim (DoubleRow rhs wants [128, 2, N])
            xt8 = const.tile([128, NCK, T], f8, name="xt8")
            dxt8 = const.tile([128, NCK, T], f8, name="dxt8")
            wv8 = const.tile([128, NCK, C], f8, name="wv8")
            dwv8 = const.tile([128, NCK, C], f8, name="dwv8")

            def _xap(base, j):
                return bass.AP(
                    tensor=xT_d, offset=base * C * T + 2 * j * 128 * T,
                    ap=[[T, 128], [128 * T, 2], [1, T]],
                )

            def _wvap(base, j):
                return bass.AP(
                    tensor=wv_d, offset=base * C * C + 2 * j * 128 * C,
                    ap=[[C, 128], [128 * C, 2], [1, C]],
                )

            for j in range(3):
                eng = nc.sync if j % 2 == 0 else nc.scalar
                eng.dma_start(out=xt8[:, 2 * j:2 * j + 2, :], in_=_xap(0, j))
                eng2 = nc.scalar if j % 2 == 0 else nc.sync
                eng2.dma_start(out=dxt8[:, 2 * j:2 * j + 2, :], in_=_xap(1, j))
            for j in range(3):
                nc.sync.dma_start(out=wv8[:, 2 * j:2 * j + 2, :],
                                  in_=_wvap(0, j))
                nc.sync.dma_start(out=dwv8[:, 2 * j:2 * j + 2, :],
                                  in_=_wvap(1, j))
            U = const.tile([128, 128], f16)
            make_upper_triangular(nc, U[:, :], val=1.0, diag=True)
            ones_f = const.tile([128, 12], f32)
            nc.vector.memset(ones_f, 1.0)
            ones_t = const.tile([128, 128], f32r)
            nc.vector.tensor_copy(
                out=ones_t, in_=ones_f[:, 0:1].broadcast_to([128, 128])
            )

            # v in token-major layout: per k-tile, 12 heads x (64 cols of v | 1)
            v_all = const.tile([128, NT, H * 65], f16)
            v_heads = v_all.rearrange("p k (h c) -> p k h c", h=H)
            for kt in range(NT):
                nc.vector.tensor_copy(
                    out=v_heads[:, kt, :, 64:65],
                    in_=ones_f.rearrange("p (h o) -> p h o", o=1),
                )

            outTs = [
                opool.tile([128, T], f16, tag=f"outT{p}", name=f"outT{p}")
                for p in range(NPAIR)
            ]
            rs3 = [
                opool.tile([128, T], f32r, tag=f"rs{j}", name=f"rs{j}")
                for j in range(3)
            ]

            # ---------- emit helpers (software-pipelined) ----------
            def emit_vproj(kts):
                # v = (x8+dx8)(wv8+dwv8) to first order: three DoubleRow
                # accumulation passes (x8*wv8 + dx8*wv8 + x8*dwv8), each
                # contracting 256 channels per instruction at 0.5 cy/col
                with nc.named_scope("vproj"):
                    for kt in kts:
                        ktsl = slice(kt * 128, (kt + 1) * 128)
                        pv = ps1.tile([128, C], f32, tag="big2", bufs=2,
                                      name="pv")
                        terms = ((xt8, wv8), (xt8, dwv8), (dxt8, wv8))
                        first, last = (0, 0), (2, 2)
                        for ti, (xs, ws) in enumerate(terms):
                            for j in range(3):
                                jsl = slice(2 * j, 2 * j + 2)
                                for n0, n1 in ((0, 512), (512, 768)):
                                    nc.tensor.matmul(
                                        pv[:, n0:n1],
                                        lhsT=xs[:, jsl, ktsl],
                                        rhs=ws[:, jsl, n0:n1],
                                        start=((ti, j) == first),
                                        stop=((ti, j) == last),
                                        perf_mode=(
                                            mybir.MatmulPerfMode.DoubleRow),
                                    )
                        nc.scalar.copy(
                            out=v_heads[:, kt, :, 0:64],
                            in_=pv.rearrange("p (h c) -> p h c", h=H),
                        )

            qkts = {}

            def qkT_group(p, wqkt, qT, kTt, pqks, gi, half):
                # one accumulation group split into two emission halves so
                # filler work drips at ~1.3us granularity; the psum tile is
                # shared across both halves.
                # q/k are computed 3-term compensated in fp8 DoubleRow
                # ((x8+dx8)(w8+dw8) to first order); q is then stored as
                # plain fp8 and k as {k8, dk8=k-k8} so the DoubleRow scores
                # matmul computes (k8+dk8)*q8 = k*q8 at half the per-column
                # cost with only q-side fp8 error
                tch = gi % 2
                tsl = slice(tch * 512, (tch + 1) * 512)
                with nc.named_scope(f"qk{p}"):
                    if half == 0:
                        pqks[gi] = ps1.tile([128, 512], f32, tag="po",
                                            bufs=4, name="pqk")
                    pqk = pqks[gi]
                    wo = (gi // 2) * 128
                    wsl = slice(wo, wo + 128)
                    terms = [(xt8, 0, j) for j in range(3)] + \
                            [(xt8, 1, j) for j in range(3)] + \
                            [(dxt8, 0, j) for j in range(3)]
                    sub = terms[:5] if half == 0 else terms[5:]
                    for (xs, wi, j) in sub:
                        jsl = slice(2 * j, 2 * j + 2)
                        nc.tensor.matmul(
                            pqk,
                            lhsT=wqkt[:, wi, jsl, wsl],
                            rhs=xs[:, jsl, tsl],
                            start=((xs is xt8) and wi == 0 and j == 0),
                            stop=((xs is dxt8) and j == 2),
                            perf_mode=mybir.MatmulPerfMode.DoubleRow,
                        )
                    if half == 1:
                        if gi // 2 == 0:
                            nc.vector.tensor_copy(out=qT[:, tsl], in_=pqk)
                        else:
                            nc.vector.tensor_copy(out=kTt[:, 0, tsl],
                                                  in_=pqk)
                            nc.vector.tensor_sub(
                                kTt[:, 1, tsl], pqk, kTt[:, 0, tsl]
                            )

            def make_qkT_fillers(p, wqkt=None):
                # allocate tiles and launch the weight DMA now; the eight
                # matmul half-groups are emitted later, dripped into exp-wait
                # bubbles of the current pair's attention
                if wqkt is None:
                    wqkt = work.tile([128, 2, NCK, 256], f8, tag="wqkt",
                                     bufs=3, name="wqkt")
                    nc.sync.dma_start(out=wqkt, in_=wqk_d[p])
                qT = work.tile([128, T], f8, tag="qT", bufs=3, name="qT")
                kTt = work.tile([128, 2, T], f8, tag="kTt", bufs=3,
                                name="kTt")
                qkts[p] = (qT, kTt)
                pqks = {}
                return [
                    (lambda gi=gi, half=half: qkT_group(
                        p, wqkt, qT, kTt, pqks, gi, half))
                    for gi in range(4)
                    for half in range(2)
                ]

            for f in make_qkT_fillers(0, wqkt0):
                f()
            fillers = []

            # ---------- weights for proj (load during phase 1) ----------
            wpts = []
            for pp in range(NPAIR):
                wpt = const.tile([128, C], f16, tag=f"wp{pp}", name=f"wp{pp}")
                wpts.append(wpt)

            proj_state = {}

            def proj_piece(tt, half):
                # one half (pairs 0-2 | 3-5) of one token tile's projection;
                # pieces are dripped into the last pair's attention bubbles
                with nc.named_scope("proj"):
                    if half == 0:
                        proj_state[tt] = ps1.tile([128, C], f32, tag="big2",
                                                  bufs=2, name="py")
                    py = proj_state[tt]
                    for p in range(3 * half, 3 * half + 3):
                        for n0, n1 in ((0, 512), (512, 768)):
                            nc.tensor.matmul(
                                py[:, n0:n1],
                                lhsT=outTs[p][:, tt * 128:(tt + 1) * 128],
                                rhs=wpts[p][:, n0:n1],
                                start=(p == 0),
                                stop=(p == NPAIR - 1),
                            )
                    if half == 1:
                        ysb = work.tile([128, C], f16, tag="ysb", bufs=3,
                                        name="ysb")
                        if tt % 2 == 0:
                            nc.vector.tensor_copy(out=ysb, in_=py)
                        else:
                            nc.scalar.copy(out=ysb, in_=py)
                        nc.sync.dma_start(
                            out=y_d[tt * 128:(tt + 1) * 128, :], in_=ysb
                        )

            def emit_proj(tts):
                for tt in tts:
                    proj_piece(tt, 0)
                    proj_piece(tt, 1)

            # ---------- phase 1: attention (qkT/vproj pipelined in) --------
            for p in range(NPAIR):
                qT, kTt = qkts.pop(p)
                if p + 1 < NPAIR:
                    fillers += make_qkT_fillers(p + 1)
                if p == 2:
                    # wp loads: late enough not to fight input DMAs,
                    # early enough to be resident before proj starts
                    for pp in range(NPAIR):
                        nc.sync.dma_start(out=wpts[pp], in_=wp_d[pp])
                with nc.named_scope(f"att{p}"):
                    for qc in range(2):
                        if p == NPAIR - 1 and qc == 1:
                            # all pairs' outT for tokens 0..511 are final:
                            # drip the first half of the projection into this
                            # window's exp-wait bubbles (the last pair has no
                            # next-pair qkT work to fill them)
                            fillers += [
                                (lambda tt=tt, half=half: proj_piece(tt, half))
                                for tt in range(4)
                                for half in range(2)
                            ]
                        nkt = 4 * (qc + 1)
                        po_pair = []
                        for hh in range(2):
                            po = ps1.tile([65, 512], f32, tag="po", bufs=4,
                                          name=f"po{hh}")
                            po_pair.append(po)
                        def scores_exp(kt):
                            # scores pair -> exp -> causal mask for one k-tile
                            ccol = max(0, 128 * kt - 512 * qc)
                            pt = ppool.tile([128, 2, 512], f16, tag="pt",
                                            bufs=6, name="pt")
                            pscr = ps1.tile([128, 2, 512], f32, tag="big2",
                                            bufs=2, name="pscr")
                            nw = 512 - ccol
                            for hh in range(2):
                                nc.tensor.matmul(
                                    pscr[:, hh, ccol:512],
                                    lhsT=kTt[hh * 64:(hh + 1) * 64, :,
                                             kt * 128:(kt + 1) * 128],
                                    rhs=qT[hh * 64:(hh + 1) * 64,
                                           qc * 512 + ccol:(qc + 1) * 512]
                                        .rearrange("p (o n) -> p o n", o=1)
                                        .broadcast_to([64, 2, nw]),
                                    start=True,
                                    stop=True,
                                    perf_mode=mybir.MatmulPerfMode.DoubleRow,
                                )
                            nc.scalar.activation(
                                out=pt[:, :, ccol:512],
                                in_=pscr[:, :, ccol:512],
                                func=AF.Exp,
                                scale=float(HS) ** -0.5 / (WSCALE * WSCALE),
                            )
                            if 128 * kt >= 512 * qc:
                                nc.gpsimd.tensor_mul(
                                    pt[:, :, ccol:ccol + 128],
                                    pt[:, :, ccol:ccol + 128],
                                    U.rearrange("p (o c) -> p o c", o=1)
                                        .broadcast_to([128, 2, 128]),
                                )
                            return pt, ccol

                        # software-pipelined: scores/exp/mask run ahead of
                        # the PV that consumes them. For pair 0 the lookahead
                        # is deep (scores need only qT/kT) and the v
                        # projection is emitted AFTER the prefill, so exp
                        # work fills the wait for the Wv/x input DMAs.
                        look = 5 if p == 0 else 2
                        # process masked (diagonal-crossing) k-tiles first so
                        # the window's last PV never waits on the exp->mask
                        # chain; the first tile in order must have ccol == 0
                        # (its start=True zeroes the full accumulator range)
                        kt_list = [0, 3, 1, 2] if qc == 0 else \
                                  [0, 5, 6, 4, 7, 1, 2, 3]
                        pts = {}
                        for kk in kt_list[:min(look, nkt)]:
                            pts[kk] = scores_exp(kk)
                        if p == 0 and qc == 0:
                            emit_vproj(range(0, 4))
                        if p == 0 and qc == 1:
                            emit_vproj(range(4, NT))
                        if fillers:
                            # independent PE work while the first exp/mask
                            # chain of this q-window completes
                            fillers.pop(0)()
                        for idx, kt in enumerate(kt_list):
                            if idx + look < nkt:
                                pts[kt_list[idx + look]] = scores_exp(
                                    kt_list[idx + look])
                            pt, ccol = pts.pop(kt)
                            for hh in range(2):
                                h = 2 * p + hh
                                nc.tensor.matmul(
                                    po_pair[hh][:, ccol:512],
                                    lhsT=v_all[:, kt, h * 65:(h + 1) * 65],
                                    rhs=pt[:, hh, ccol:512],
                                    start=(idx == 0),
                                    stop=(idx == nkt - 1),
                                )
                            if fillers and idx % 2 == 1:
                                fillers.pop(0)()
                        qsl = slice(qc * 512, (qc + 1) * 512)
                        # reciprocals first (they gate the pr matmuls on PE)
                        for hh in range(2):
                            h = 2 * p + hh
                            with nc.allow_low_precision(
                                reason="1/rowsum at fp32r (12-bit mantissa) "
                                       "costs ~1e-4 relative error"
                            ):
                                nc.vector.reciprocal(
                                    out=rs3[h // 4][(h % 4) * 32:
                                                    (h % 4) * 32 + 1, qsl],
                                    in_=po_pair[hh][64:65, :],
                                )
                        for hh in range(2):
                            # split across engines so both po banks free in
                            # parallel
                            if hh == 0:
                                nc.scalar.copy(
                                    out=outTs[p][0:64, qsl],
                                    in_=po_pair[0][0:64, :],
                                )
                            else:
                                nc.vector.tensor_copy(
                                    out=outTs[p][64:128, qsl],
                                    in_=po_pair[1][0:64, :],
                                )
                        # normalize: broadcast each 1/rowsum row across
                        # 64 partitions. For pairs 0-4 this goes via a DRAM
                        # round-trip DMA (stride-0 partition reads from DRAM
                        # are legal) -- higher latency but off the PE and off
                        # the contended po psum slots; outT isn't needed
                        # until the projection anyway. The last pair gates
                        # the projection tail, so it keeps the low-latency
                        # K=1 matmul path.
                        if p < NPAIR - 1:
                            prsb = work.tile([128, 512], f32r, tag="prsb",
                                             bufs=3, name="prsb")
                            for hh in range(2):
                                h = 2 * p + hh
                                r0 = (h % 4) * 32
                                prd = dscr.tile([1, 512], f32r, tag="prd",
                                                bufs=4, name="prd")
                                nc.sync.dma_start(
                                    out=prd[0:1, :],
                                    in_=rs3[h // 4][r0:r0 + 1, qsl],
                                )
                                nc.sync.dma_start(
                                    out=prsb[hh * 64:(hh + 1) * 64, :],
                                    in_=prd[0:1, :].broadcast_to([64, 512]),
                                )
                            nc.vector.tensor_mul(
                                outTs[p][:, qsl],
                                outTs[p][:, qsl],
                                prsb,
                            )
                        else:
                            for hh in range(2):
                                h = 2 * p + hh
                                r0 = (h % 4) * 32
                                pr = ps1.tile([128, 512], f32, tag="po",
                                              bufs=4, name="pr")
                                nc.tensor.matmul(
                                    pr,
                                    lhsT=ones_t[r0:r0 + 1, :],
                                    rhs=rs3[h // 4][r0:r0 + 1, qsl],
                                    start=True,
                                    stop=True,
                                    tile_position=(r0, 0),
                                )
                                nc.vector.tensor_mul(
                                    outTs[p][hh * 64:(hh + 1) * 64, qsl],
                                    outTs[p][hh * 64:(hh + 1) * 64, qsl],
                                    pr[hh * 64:(hh + 1) * 64, :],
                                )

            for f in fillers:
                f()
            fillers = []

            # ---------- phase 2: rest of the output projection ----------
            emit_proj(range(4, NT))

    nc.compile()
    return nc


def get_nc():
    global _cached_nc
    if _cached_nc is None:
        _cached_nc = _build_nc()
    return _cached_nc


def _f8_pair(a):
    """Quantize to fp8 e4m3 with a first-order residual: returns (a8, da8)
    stacked on axis 0 so a8 + da8 ~= a to ~0.2%."""
    import ml_dtypes

    f8 = ml_dtypes.float8_e4m3
    a = np.asarray(a, np.float32)
    a8 = a.astype(f8)
    da8 = (a - a8.astype(np.float32)).astype(f8)
    return np.stack([a8, da8])


def _host_pack(inputs):
    x = np.asarray(inputs["x"], dtype=np.float32)
    Wq = np.asarray(inputs["Wq"], dtype=np.float32)
    Wk = np.asarray(inputs["Wk"], dtype=np.float32)
    Wv = np.asarray(inputs["Wv"], dtype=np.float32)
    Wproj = np.asarray(inputs["Wproj"], dtype=np.float32)

    Wq2 = Wq.transpose(1, 0, 2).reshape(C, C)  # [c, h*HS]
    Wk2 = Wk.transpose(1, 0, 2).reshape(C, C)
    wqk = np.stack(
        [
            np.concatenate(
                [
                    Wq2[:, p * 128:(p + 1) * 128],
                    Wk2[:, p * 128:(p + 1) * 128],
                ],
                axis=1,
            )
            for p in range(NPAIR)
        ]
    )  # [6, 768(c), 256]
    # -> [6, 128(kp), 6(ck), 256] so the on-chip tile loads contiguously
    wqk = np.ascontiguousarray(
        wqk.reshape(NPAIR, NCK, 128, 256).transpose(0, 2, 1, 3)
    ) * WSCALE
    wqk = np.ascontiguousarray(
        _f8_pair(wqk).transpose(1, 2, 0, 3, 4)
    )  # [6, 128(kp), 2, 6, 256]
    wv = _f8_pair(
        np.ascontiguousarray(Wv.transpose(1, 0, 2).reshape(C, C)) * WSCALE
    )  # [2, C, C]
    wp = np.ascontiguousarray(
        Wproj.T.reshape(NPAIR, 128, C) / WSCALE
    ).astype(np.float16)
    shared = {"wqk": wqk, "wv": wv, "wp": wp}
    in_maps = [
        dict(shared, xT=_f8_pair(np.ascontiguousarray(x[b].T)))
        for b in range(B)
    ]
    return in_maps


def kernel(**inputs):
    global LAST_EXEC_NS, LAST_RESULTS
    from concourse.bass_utils import run_bass_kernel_spmd

    nc = get_nc()
    in_maps = _host_pack(inputs)
    trace = bool(int(os.environ.get("KERNEL_TRACE", "0")))
    res = run_bass_kernel_spmd(
        nc, in_maps, core_ids=list(range(B)), trace=trace
    )
    LAST_EXEC_NS = res.exec_time_ns
    LAST_RESULTS = res
    bproj = np.asarray(inputs["bproj"], dtype=np.float32)
    y = np.stack([res.results[b]["y"] for b in range(B)])
    return y.astype(np.float32) + bproj
